# revision 1
# baseline (speedup 1.0000x reference)
"""Talking-heads attention with GFSA reaction term — TRN2 Bass kernel, 8 cores.

Sharding: (batch b, query-half) -> core c = b*2 + half. Each core handles all 12
heads for its 512 query rows; the key/value axis m stays full but is ROTATED on
the host so m-tiles 0..3 are the core's own query rows ("local") and 4..7 the
partner's. The only cross-core dependency is w1 = attn3 @ v over the full query
axis, exchanged with a per-pair AllGather (1.5MB) that overlaps pass-2's
local-half matmuls; the remote half is extracted with a host-fed 0/1 mask blend
so the program stays rank-symmetric.

Math (associativity rewrite — never materialize attn3 @ attn3):
  attn2[g]  = sum_h W1[g,h] (q_h*SCALE) @ k_h^T + b1[g]   (folded into QK^T)
  s_g       = softmax_m(attn2[g])  (E=exp stored fp16 [m,n] layout, normalized
                                    in place by broadcast 1/Z; no max-subtract
                                    needed: |scores| <~ 6)
  attn3[g'] = sum_g W2[g',g] s_g + b2[g']                  (folded into A@v)
  w1[g']    = attn3[g'] @ v_g'
  out[g']   = attn3[g'] @ ((1-2*lam)v_g' + 3*lam*w1[g'])
  y         = out @ Wo^T + ob
Both passes accumulate all 12 g and all m-tiles into 8 long-lived PSUM bank
groups (one per 128x384-ish output region), so each output region has a single
consume op — no per-head consume chains.
"""
import numpy as np

import concourse.bacc as bacc
import concourse.mybir as mybir
import concourse.tile as tile
from concourse.bass_utils import run_bass_kernel_spmd
from concourse.masks import make_identity

B, N, DIM, HEADS, HD = 4, 1024, 768, 12, 64
NH = N // 2                      # query rows per core
SCALE = HD ** -0.5
f32 = mybir.dt.float32
f32r = mybir.dt.float32r
f16 = mybir.dt.float16
AL = mybir.AluOpType
AF = mybir.ActivationFunctionType

TRACE = False                    # test.py may flip this for profiling
TRACE_KW = {}
DEBUG = False                    # dump intermediates as extra outputs


def _build():
    nc = bacc.Bacc("TRN2", target_bir_lowering=False, debug=False, num_devices=8)

    xf_T = nc.declare_dram_parameter("xf_T", [DIM, N], f32r, isOutput=False)
    wq_T = nc.declare_dram_parameter("wq_T", [DIM, DIM], f32r, isOutput=False)
    wk_T = nc.declare_dram_parameter("wk_T", [DIM, DIM], f32r, isOutput=False)
    wv_T = nc.declare_dram_parameter("wv_T", [DIM, DIM], f32r, isOutput=False)
    wo_T = nc.declare_dram_parameter("wo_T", [DIM, DIM], f32r, isOutput=False)
    w1v = nc.declare_dram_parameter("w1v", [128, 72], f32, isOutput=False)
    b1bc = nc.declare_dram_parameter("b1bc", [128, HEADS], f32, isOutput=False)
    w2f = nc.declare_dram_parameter("w2f", [1, HEADS * DIM], f32, isOutput=False)
    uc1 = nc.declare_dram_parameter("uc1", [1, DIM], f32, isOutput=False)
    uc2 = nc.declare_dram_parameter("uc2", [1, DIM], f32, isOutput=False)
    b2blk = nc.declare_dram_parameter("b2blk", [1, DIM], f32, isOutput=False)
    ob = nc.declare_dram_parameter("ob", [1, DIM], f32, isOutput=False)
    msk = nc.declare_dram_parameter("msk", [128, 2], f32, isOutput=False)
    y = nc.declare_dram_parameter("y", [NH, DIM], f32, isOutput=True)
    if DEBUG:
        dbg_E = nc.declare_dram_parameter("dbg_E", [128, 8, HEADS, NH], f16,
                                          isOutput=True)
        dbg_w1 = nc.declare_dram_parameter("dbg_w1", [128, 4, DIM], f32, isOutput=True)
        dbg_u = nc.declare_dram_parameter("dbg_u", [128, 8, DIM], f16, isOutput=True)
        dbg_acc2 = nc.declare_dram_parameter("dbg_acc2", [128, 4, DIM], f32,
                                             isOutput=True)
        dbg_outT = nc.declare_dram_parameter("dbg_outT", [128, 6, NH], f16,
                                             isOutput=True)

    with tile.TileContext(nc) as tc:
        with tc.tile_pool(name="persist", bufs=1) as pp:
            # [m%128, m//128, g, n_local] fp16 probabilities (transposed layout);
            # written as exp(scores), then normalized in place.
            E = pp.tile([128, 8, HEADS, NH], f16)
            v16 = pp.tile([128, 8, DIM], f16)          # [m%128, m//128, (g',d)]
            w1v_sb = pp.tile([128, 72], f32)
            b1_sb = pp.tile([128, HEADS], f32)
            uc1bc = pp.tile([128, DIM], f16)
            uc2bc = pp.tile([128, DIM], f16)
            b2bc = pp.tile([128, DIM], f16)
            msk_sb = pp.tile([128, 2], f32)
            ones128 = pp.tile([128, 128], f16)
            b2v = pp.tile([128, DIM], f32)
            w2all = pp.tile([128, HEADS, DIM], f16)
            nc.sync.dma_start(w1v_sb[:], w1v[:])
            nc.sync.dma_start(b1_sb[:], b1bc[:])
            nc.sync.dma_start(msk_sb[:], msk[:])
            nc.gpsimd.dma_start(b2bc[:], b2blk[0:1, :].to_broadcast((128, DIM)))
            nc.vector.memset(ones128[:], 1.0)

            with tc.tile_pool(name="qk", bufs=1) as qk:
                qT = qk.tile([128, 6, NH], f16)        # [d%128, d//128, n_local]
                kT = qk.tile([128, 6, N], f16)         # [d%128, d//128, m]

                # ---- Phase A: QKV projections -------------------------------
                with tc.tile_pool(name="pha", bufs=1) as pa, \
                     tc.tile_pool(name="wsl", bufs=4) as wsl, \
                     tc.tile_pool(name="wvr", bufs=1) as wvr, \
                     tc.tile_pool(name="psa", bufs=2, space="PSUM") as psa, \
                     tc.tile_pool(name="psav", bufs=1, space="PSUM") as psav:
                    xf = pa.tile([128, 6, N], f32r)
                    xfr = xf_T.rearrange("(c p) n -> p c n", p=128)
                    for d in range(6):
                        nc.gpsimd.dma_start(xf[:, d, 0:NH], xfr[:, d, 0:NH])
                    for d in range(6):
                        nc.gpsimd.dma_start(xf[:, d, NH:N], xfr[:, d, NH:N])
                    wqr = wq_T.rearrange("(c p) n -> p c n", p=128)
                    wkr = wk_T.rearrange("(c p) n -> p c n", p=128)
                    wvr_ap = wv_T.rearrange("(c p) n -> p c n", p=128)

                    for qc in range(6):                # qT[c, n] = sum_d wq[d,c]x[n,d]
                        ps = psa.tile([128, NH], f32, tag="ps512")
                        for d in range(6):
                            wsl_t = wsl.tile([128, 128], f32r, tag="w")
                            nc.sync.dma_start(wsl_t[:], wqr[:, d, qc * 128:(qc + 1) * 128])
                            nc.tensor.matmul(ps[:], wsl_t[:], xf[:, d, 0:NH],
                                             start=(d == 0), stop=(d == 5))
                        nc.vector.tensor_copy(qT[:, qc, :], ps[:])
                    for kc in range(6):
                        for mc in range(2):
                            ps = psa.tile([128, 512], f32, tag="ps512")
                            for d in range(6):
                                wsl_t = wsl.tile([128, 128], f32r, tag="w")
                                nc.sync.dma_start(wsl_t[:],
                                                  wkr[:, d, kc * 128:(kc + 1) * 128])
                                nc.tensor.matmul(ps[:], wsl_t[:],
                                                 xf[:, d, mc * 512:(mc + 1) * 512],
                                                 start=(d == 0), stop=(d == 5))
                            nc.vector.tensor_copy(kT[:, kc, mc * 512:(mc + 1) * 512],
                                                  ps[:])
                    for vc in range(2):                # v[m, c] = sum_d x[m,d]wv[d,c]
                        wv_t = wvr.tile([128, 6, 384], f32r, tag="wv")
                        for d in range(6):
                            nc.sync.dma_start(wv_t[:, d, :],
                                              wvr_ap[:, d, vc * 384:(vc + 1) * 384])
                        for mt in range(8):
                            ps = psa.tile([128, 384], f32, tag="ps384")
                            for d in range(6):
                                nc.tensor.matmul(ps[:], xf[:, d, mt * 128:(mt + 1) * 128],
                                                 wv_t[:, d, :],
                                                 start=(d == 0), stop=(d == 5))
                            nc.vector.tensor_copy(v16[:, mt, vc * 384:(vc + 1) * 384],
                                                  ps[:])
                    # b2v = b2blk * colsum(v), row-replicated via all-ones lhsT
                    psV = psav.tile([128, 2, 512], f32)
                    for half in range(2):
                        for mt in range(8):
                            nc.tensor.matmul(psV[:, half, 0:384],
                                             ones128[:],
                                             v16[:, mt, half * 384:(half + 1) * 384],
                                             start=(mt == 0), stop=(mt == 7))
                    nc.vector.tensor_tensor(b2v[:, 0:384], psV[:, 0, 0:384],
                                            b2bc[:, 0:384], AL.mult)
                    nc.vector.tensor_tensor(b2v[:, 384:768], psV[:, 1, 0:384],
                                            b2bc[:, 384:768], AL.mult)

                # ---- Phase B: mixed scores (mix1 fold), exp, Z, normalize ---
                with tc.tile_pool(name="qsc", bufs=2) as qscp, \
                     tc.tile_pool(name="zbc", bufs=2) as zbcp, \
                     tc.tile_pool(name="zdr", bufs=2, space="DRAM") as zdrp, \
                     tc.tile_pool(name="psb", bufs=4, space="PSUM") as psb, \
                     tc.tile_pool(name="psz", bufs=2, space="PSUM") as psz:
                    for g in range(HEADS):
                        qsc = qscp.tile([128, 6, NH], f16, tag="qsc")
                        for i in range(6):
                            nc.scalar.activation(qsc[:, i, :], qT[:, i, :], AF.Copy,
                                                 scale=w1v_sb[:, g * 6 + i:g * 6 + i + 1])
                        for mt in range(8):
                            ps = psb.tile([128, NH], f32, tag="psb")
                            for i in range(6):
                                nc.tensor.matmul(ps[:], kT[:, i, mt * 128:(mt + 1) * 128],
                                                 qsc[:, i, :], start=(i == 0),
                                                 stop=(i == 5))
                            nc.scalar.activation(E[:, mt, g, :], ps[:], AF.Exp,
                                                 bias=b1_sb[:, g:g + 1], scale=1.0)
                        # Zrow_g[n] = sum_m E_g[m, n]  (free-axis layout, M=1)
                        psZ = psz.tile([1, NH], f32, tag="psz")
                        for mt in range(8):
                            nc.tensor.matmul(psZ[0:1, :], ones128[:, 0:1],
                                             E[:, mt, g, :], start=(mt == 0),
                                             stop=(mt == 7))
                        ztmp = zbcp.tile([1, NH], f32, tag="zt")
                        nc.vector.reciprocal(ztmp[0:1, :], psZ[0:1, :])
                        # broadcast 1/Z to all partitions (fp16) via a DRAM
                        # bounce (SBUF APs cannot have zero partition step)
                        zdr = zdrp.tile([1, NH], f32, tag="zdr")
                        nc.sync.dma_start(zdr[:], ztmp[0:1, :])
                        zb = zbcp.tile([128, NH], f16, tag="zb")
                        nc.gpsimd.dma_start(zb[:], zdr[0:1, :].to_broadcast((128, NH)))
                        for mt in range(8):
                            nc.vector.tensor_tensor(E[:, mt, g, :], E[:, mt, g, :],
                                                    zb[:], AL.mult)

            nc.gpsimd.dma_start(uc1bc[:], uc1[0:1, :].to_broadcast((128, DIM)))
            nc.gpsimd.dma_start(uc2bc[:], uc2[0:1, :].to_broadcast((128, DIM)))
            for g in range(HEADS):
                nc.gpsimd.dma_start(
                    w2all[:, g, :],
                    w2f[0:1, g * DIM:(g + 1) * DIM].to_broadcast((128, DIM)))

            if DEBUG:
                nc.sync.dma_start(dbg_E[:], E[:])

            with tc.tile_pool(name="late", bufs=1) as late:
                w1acc = late.tile([128, 4, DIM], f32)
                acc = late.tile([128, 4, DIM], f32)    # pass-2 accumulator
                u16L = late.tile([128, 4, DIM], f16)
                u16R = late.tile([128, 4, DIM], f16)

                # ---- Pass 1: w1 = attn3 @ v (mix2 folded; 8 PSUM bank groups
                #      accumulate over all g and m) ---------------------------
                with tc.tile_pool(name="vt", bufs=2) as vtp, \
                     tc.tile_pool(name="psc", bufs=4, space="PSUM") as psc:
                    psAs = [psc.tile([128, 512], f32, tag="pscA", name=f"pscA{i}") for i in range(4)]
                    psBs = [psc.tile([128, 256], f32, tag="pscB", name=f"pscB{i}") for i in range(4)]
                    for g in range(HEADS):
                        Vt = vtp.tile([128, 8, DIM], f16, tag="vt")
                        for mt in range(8):
                            nc.vector.tensor_tensor(Vt[:, mt, :], v16[:, mt, :],
                                                    w2all[:, g, :], AL.mult)
                        for ns in range(4):
                            for mt in range(8):
                                lhs = E[:, mt, g, ns * 128:(ns + 1) * 128]
                                first = (g == 0 and mt == 0)
                                last = (g == HEADS - 1 and mt == 7)
                                nc.tensor.matmul(psAs[ns][:], lhs, Vt[:, mt, 0:512],
                                                 start=first, stop=last)
                                nc.tensor.matmul(psBs[ns][:], lhs, Vt[:, mt, 512:768],
                                                 start=first, stop=last)
                    for ns in range(4):
                        nc.vector.tensor_tensor(w1acc[:, ns, 0:512], psAs[ns][:],
                                                b2v[:, 0:512], AL.add)
                        nc.vector.tensor_tensor(w1acc[:, ns, 512:768], psBs[ns][:],
                                                b2v[:, 512:768], AL.add)

                if DEBUG:
                    nc.sync.dma_start(dbg_w1[:], w1acc[:])

                # ---- AllGather w1 + pass 2: out = attn3 @ u.
                # Order matters: the collective fires first; u16L/UtL builds and
                # the local-half matmuls depend only on w1acc, so PE/DVE work
                # fills the gather latency. The u16R blend + remote-half
                # matmuls are emitted after all local work so the DVE stream
                # never head-of-line blocks on the collective.
                with tc.tile_pool(name="dram", bufs=1, space="DRAM") as dram, \
                     tc.tile_pool(name="w1g", bufs=1) as w1gp, \
                     tc.tile_pool(name="ut", bufs=2) as utp, \
                     tc.tile_pool(name="pse", bufs=4, space="PSUM") as pse:
                    w1loc = dram.tile([NH, DIM], f16)
                    w1full = dram.tile([N, DIM], f16)
                    nc.gpsimd.dma_start(
                        w1loc.rearrange("(ns p) j -> p ns j", p=128), w1acc[:])
                    nc.gpsimd.collective_compute(
                        "AllGather", AL.bypass,
                        replica_groups=[[0, 1], [2, 3], [4, 5], [6, 7]],
                        ins=[w1loc.opt()], outs=[w1full.opt()])
                    # local half of u: no collective dependency
                    for j in range(4):
                        t1 = w1gp.tile([128, DIM], f16, tag="ub1")
                        t2 = w1gp.tile([128, DIM], f16, tag="ub2")
                        nc.vector.tensor_tensor(t1[:], v16[:, j, :], uc1bc[:], AL.mult)
                        nc.vector.tensor_tensor(t2[:], w1acc[:, j, :], uc2bc[:], AL.mult)
                        nc.vector.tensor_add(u16L[:, j, :], t1[:], t2[:])
                    psAs = [pse.tile([128, 512], f32, tag="pseA", name=f"pseA{i}")
                            for i in range(4)]
                    psBs = [pse.tile([128, 256], f32, tag="pseB", name=f"pseB{i}")
                            for i in range(4)]
                    for g in range(HEADS):
                        UtL = utp.tile([128, 4, DIM], f16, tag="ut")
                        for j in range(4):
                            nc.vector.tensor_tensor(UtL[:, j, :], u16L[:, j, :],
                                                    w2all[:, g, :], AL.mult)
                        for ns in range(4):
                            for j in range(4):
                                lhs = E[:, j, g, ns * 128:(ns + 1) * 128]
                                first = (g == 0 and j == 0)
                                nc.tensor.matmul(psAs[ns][:], lhs, UtL[:, j, 0:512],
                                                 start=first, stop=False)
                                nc.tensor.matmul(psBs[ns][:], lhs, UtL[:, j, 512:768],
                                                 start=first, stop=False)
                    # remote half: mask-blend of the two gathered blocks
                    w1fr = w1full.rearrange("(mt p) j -> p mt j", p=128)
                    w1b = w1gp.tile([128, 2, 4, DIM], f16)
                    nc.sync.dma_start(w1b[:, 0, :, :], w1fr[:, 0:4, :])
                    nc.sync.dma_start(w1b[:, 1, :, :], w1fr[:, 4:8, :])
                    for j in range(4):
                        tr = w1gp.tile([128, DIM], f16, tag="ub3")
                        t3 = w1gp.tile([128, DIM], f16, tag="ub4")
                        nc.vector.tensor_scalar(tr[:], w1b[:, 0, j, :],
                                                msk_sb[:, 0:1], None, AL.mult)
                        nc.vector.tensor_scalar(t3[:], w1b[:, 1, j, :],
                                                msk_sb[:, 1:2], None, AL.mult)
                        nc.vector.tensor_add(tr[:], tr[:], t3[:])
                        nc.vector.tensor_tensor(tr[:], tr[:], uc2bc[:], AL.mult)
                        t1 = w1gp.tile([128, DIM], f16, tag="ub1")
                        nc.vector.tensor_tensor(t1[:], v16[:, 4 + j, :], uc1bc[:],
                                                AL.mult)
                        nc.vector.tensor_add(u16R[:, j, :], t1[:], tr[:])
                    if DEBUG:
                        nc.sync.dma_start(dbg_u[:, 0:4, :], u16L[:])
                        nc.sync.dma_start(dbg_u[:, 4:8, :], u16R[:])
                    for g in range(HEADS):
                        UtR = utp.tile([128, 4, DIM], f16, tag="ut")
                        for j in range(4):
                            nc.vector.tensor_tensor(UtR[:, j, :], u16R[:, j, :],
                                                    w2all[:, g, :], AL.mult)
                        for ns in range(4):
                            for j in range(4):
                                lhs = E[:, 4 + j, g, ns * 128:(ns + 1) * 128]
                                last = (g == HEADS - 1 and j == 3)
                                nc.tensor.matmul(psAs[ns][:], lhs, UtR[:, j, 0:512],
                                                 start=False, stop=last)
                                nc.tensor.matmul(psBs[ns][:], lhs, UtR[:, j, 512:768],
                                                 start=False, stop=last)
                    for ns in range(4):
                        nc.vector.tensor_copy(acc[:, ns, 0:512], psAs[ns][:])
                        nc.vector.tensor_copy(acc[:, ns, 512:768], psBs[ns][:])

                # + b2[g'] * colsum(u)  (after the 8 groups are consumed)
                with tc.tile_pool(name="w2p3", bufs=1) as w2p3, \
                     tc.tile_pool(name="psev", bufs=1, space="PSUM") as psev:
                    psU = psev.tile([128, 2, 512], f32)
                    for j in range(4):
                        nc.tensor.matmul(psU[:, 0, :], ones128[:], u16L[:, j, 0:512],
                                         start=(j == 0), stop=False)
                        nc.tensor.matmul(psU[:, 1, 0:256], ones128[:],
                                         u16L[:, j, 512:768],
                                         start=(j == 0), stop=False)
                    for j in range(4):
                        nc.tensor.matmul(psU[:, 0, :], ones128[:], u16R[:, j, 0:512],
                                         start=False, stop=(j == 3))
                        nc.tensor.matmul(psU[:, 1, 0:256], ones128[:],
                                         u16R[:, j, 512:768],
                                         start=False, stop=(j == 3))
                    b2u = w2p3.tile([128, DIM], f32)
                    nc.vector.tensor_tensor(b2u[:, 0:512], psU[:, 0, :],
                                            b2bc[:, 0:512], AL.mult)
                    nc.vector.tensor_tensor(b2u[:, 512:768], psU[:, 1, 0:256],
                                            b2bc[:, 512:768], AL.mult)
                    for ns in range(4):
                        nc.vector.tensor_add(acc[:, ns, :], acc[:, ns, :], b2u[:])

                if DEBUG:
                    nc.sync.dma_start(dbg_acc2[:], acc[:])

                # ---- Phase F: output projection -----------------------------
                with tc.tile_pool(name="phf", bufs=1) as pf, \
                     tc.tile_pool(name="wos", bufs=6) as wos, \
                     tc.tile_pool(name="ypool", bufs=2) as ypool:
                    ident = pf.tile([128, 128], f32)
                    make_identity(nc, ident[:])
                    obbc = pf.tile([128, DIM], f16)
                    nc.gpsimd.dma_start(obbc[:], ob[0:1, :].to_broadcast((128, DIM)))
                    wor = wo_T.rearrange("(c p) n -> p c n", p=128)
                    outT = pf.tile([128, 6, NH], f16)
                    with tc.tile_pool(name="psft", bufs=4, space="PSUM") as psft:
                        for ns in range(4):
                            for jc in range(6):
                                psT = psft.tile([128, 128], f32, tag="psT")
                                nc.tensor.transpose(psT[:],
                                                    acc[:, ns, jc * 128:(jc + 1) * 128],
                                                    ident[:])
                                nc.vector.tensor_copy(
                                    outT[:, jc, ns * 128:(ns + 1) * 128], psT[:])
                    if DEBUG:
                        nc.sync.dma_start(dbg_outT[:], outT[:])
                    yr = y.rearrange("(ns p) j -> p ns j", p=128)
                    wo_ts = []
                    for jc in range(6):
                        wo_t = wos.tile([128, DIM], f16, tag="wo", name=f"wo{jc}")
                        nc.gpsimd.dma_start(wo_t[:], wor[:, jc, :].bitcast(f32))
                        wo_ts.append(wo_t)
                    with tc.tile_pool(name="psf", bufs=2, space="PSUM") as psf:
                        for ns in range(4):
                            psY = psf.tile([128, 512], f32, tag="psY")
                            psY2 = psf.tile([128, 512], f32, tag="psY2")
                            for jc in range(6):
                                nc.tensor.matmul(psY[:, :],
                                                 outT[:, jc, ns * 128:(ns + 1) * 128],
                                                 wo_ts[jc][:, 0:512], start=(jc == 0),
                                                 stop=(jc == 5))
                                nc.tensor.matmul(psY2[:, 0:256],
                                                 outT[:, jc, ns * 128:(ns + 1) * 128],
                                                 wo_ts[jc][:, 512:768], start=(jc == 0),
                                                 stop=(jc == 5))
                            y_sb = ypool.tile([128, DIM], f32, tag="ysb")
                            nc.vector.tensor_tensor(y_sb[:, 0:512], psY[:, :],
                                                    obbc[:, 0:512], AL.add)
                            nc.vector.tensor_tensor(y_sb[:, 512:768], psY2[:, 0:256],
                                                    obbc[:, 512:768], AL.add)
                            nc.sync.dma_start(yr[:, ns, :], y_sb[:])

    nc.compile()
    return nc


def kernel(x, qkv_w, proj_l_w, proj_l_b, proj_w_w, proj_w_b, lamb,
           proj_out_w, proj_out_b):
    x = np.asarray(x, dtype=np.float32)
    qkv_w = np.asarray(qkv_w, dtype=np.float32)
    proj_l_w = np.asarray(proj_l_w, dtype=np.float32)
    proj_l_b = np.asarray(proj_l_b, dtype=np.float32)
    proj_w_w = np.asarray(proj_w_w, dtype=np.float32)
    proj_w_b = np.asarray(proj_w_b, dtype=np.float32)
    lamb = np.asarray(lamb, dtype=np.float32)
    proj_out_w = np.asarray(proj_out_w, dtype=np.float32)
    proj_out_b = np.asarray(proj_out_b, dtype=np.float32)

    nc = _build()

    wq_T = np.ascontiguousarray(qkv_w[:DIM].T) * np.float32(SCALE)
    wk_T = np.ascontiguousarray(qkv_w[DIM:2 * DIM].T)
    wv_T = np.ascontiguousarray(qkv_w[2 * DIM:].T)
    wo_T = np.ascontiguousarray(proj_out_w.T)

    w1v = np.empty((128, 72), dtype=np.float32)
    for g in range(HEADS):
        for i in range(6):
            w1v[:64, g * 6 + i] = proj_l_w[g, 2 * i]
            w1v[64:, g * 6 + i] = proj_l_w[g, 2 * i + 1]
    b1bc = np.tile(proj_l_b[None, :], (128, 1)).astype(np.float32)
    # w2f[0, g*768 + g'*64 + d] = proj_w_w[g', g]
    w2f = np.repeat(proj_w_w.T, HD, axis=1).reshape(1, HEADS * DIM).astype(np.float32)
    uc1 = np.repeat(1.0 - 2.0 * lamb, HD)[None, :].astype(np.float32)
    uc2 = np.repeat(3.0 * lamb, HD)[None, :].astype(np.float32)
    b2blk = np.repeat(proj_w_b, HD)[None, :].astype(np.float32)
    ob = proj_out_b[None, :].astype(np.float32)

    in_maps = []
    for c in range(8):
        b, half = c // 2, c % 2
        # m-axis rotated: rows [0:512] are this core's own query rows
        xr = np.concatenate([x[b, half * NH:(half + 1) * NH, :],
                             x[b, (1 - half) * NH:(2 - half) * NH, :]], axis=0)
        mskv = np.empty((128, 2), dtype=np.float32)
        mskv[:, 0] = float(half)        # weight for gathered block 0 (= rank 0)
        mskv[:, 1] = float(1 - half)    # weight for gathered block 1 (= rank 1)
        in_maps.append({
            "xf_T": np.ascontiguousarray(xr.T),
            "wq_T": wq_T, "wk_T": wk_T, "wv_T": wv_T, "wo_T": wo_T,
            "w1v": w1v, "b1bc": b1bc, "w2f": w2f,
            "uc1": uc1, "uc2": uc2, "b2blk": b2blk, "ob": ob, "msk": mskv,
        })

    res = run_bass_kernel_spmd(nc, in_maps, core_ids=list(range(8)),
                               trace=TRACE, **TRACE_KW)
    kernel.last_results = res
    kernel.last_nc = nc
    kernel.last_in_maps = in_maps

    out = np.empty((B, N, DIM), dtype=np.float32)
    for c in range(8):
        b, half = c // 2, c % 2
        out[b, half * NH:(half + 1) * NH, :] = res.results[c]["y"]
    return out



# revision 5
# speedup vs baseline: 1.0309x; 1.0309x over previous
"""Talking-heads attention with GFSA reaction term — TRN2 Bass kernel, 8 cores.

Sharding: (batch b, query-half) -> core c = b*2 + half. Each core handles all 12
heads for its 512 query rows; the key/value axis m stays full but is ROTATED on
the host so m-tiles 0..3 are the core's own query rows ("local") and 4..7 the
partner's. The only cross-core dependency is u = (1-2lam)v + 3lam*(attn3 @ v)
over the full query axis, exchanged with a per-pair AllGather (1.5MB) that
overlaps pass-2's local-half matmuls; the remote half is extracted with a
host-fed 0/1 mask blend so the program stays rank-symmetric.

Math (associativity rewrite — never materialize attn3 @ attn3):
  attn2[g]  = sum_h W1[g,h] (q_h*SCALE) @ k_h^T + b1[g]   (folded into QK^T)
  s_g       = softmax_m(attn2[g])  (E=exp stored fp16 [m,n] layout, normalized
                                    in place by broadcast 1/Z; no max-subtract
                                    needed: |scores| <~ 6)
  attn3[g'] = sum_g W2[g',g] s_g + b2[g']                  (folded into A@v)
  w1[g']    = attn3[g'] @ v_g'
  out[g']   = attn3[g'] @ ((1-2*lam)v_g' + 3*lam*w1[g'])
  y         = out @ Wo^T + ob
Pass 1 accumulates all 12 g and all m-tiles into 8 long-lived PSUM bank groups
([n, d] orientation, feeds the u build). Pass 2 runs TRANSPOSED ([d, n]
orientation: lhsT=Ut, rhs=E) into 6 PSUM banks so the output-projection needs
no PE transposes. A junk-matmul warmup burst at t=0 overlaps the input DMAs
and lifts the PE HAM clock gate to 8/8 before real work starts.
"""
import numpy as np

import concourse.bacc as bacc
import concourse.mybir as mybir
import concourse.tile as tile
from concourse.bass_utils import run_bass_kernel_spmd

B, N, DIM, HEADS, HD = 4, 1024, 768, 12, 64
NH = N // 2                      # query rows per core
SCALE = HD ** -0.5
f32 = mybir.dt.float32
f16 = mybir.dt.float16
AL = mybir.AluOpType
AF = mybir.ActivationFunctionType

TRACE = False                    # test.py may flip this for profiling
TRACE_KW = {}


def _build():
    nc = bacc.Bacc("TRN2", target_bir_lowering=False, debug=False, num_devices=8)

    xf_T = nc.declare_dram_parameter("xf_T", [DIM, N], f16, isOutput=False)
    wq_T = nc.declare_dram_parameter("wq_T", [DIM, DIM], f16, isOutput=False)
    wk_T = nc.declare_dram_parameter("wk_T", [DIM, DIM], f16, isOutput=False)
    wv_T = nc.declare_dram_parameter("wv_T", [DIM, DIM], f16, isOutput=False)
    wo_T = nc.declare_dram_parameter("wo_T", [DIM, DIM], f16, isOutput=False)
    w1v = nc.declare_dram_parameter("w1v", [128, 72], f32, isOutput=False)
    b1bc = nc.declare_dram_parameter("b1bc", [128, HEADS], f32, isOutput=False)
    w2f = nc.declare_dram_parameter("w2f", [1, HEADS * DIM], f32, isOutput=False)
    uc1 = nc.declare_dram_parameter("uc1", [1, DIM], f32, isOutput=False)
    uc2 = nc.declare_dram_parameter("uc2", [1, DIM], f32, isOutput=False)
    b2blk = nc.declare_dram_parameter("b2blk", [1, DIM], f32, isOutput=False)
    ob = nc.declare_dram_parameter("ob", [1, DIM], f32, isOutput=False)
    msk = nc.declare_dram_parameter("msk", [128, 2], f32, isOutput=False)
    y = nc.declare_dram_parameter("y", [NH, DIM], f32, isOutput=True)
    warm_out = nc.declare_dram_parameter("warm_out", [1, 8], f32, isOutput=True)

    with tile.TileContext(nc) as tc:
        with tc.tile_pool(name="persist", bufs=1) as pp:
            # [m%128, m//128, g, n_local] fp16 probabilities (transposed layout);
            # written as exp(scores), then normalized in place.
            E = pp.tile([128, 8, HEADS, NH], f16)
            v16 = pp.tile([128, 8, DIM], f16)          # [m%128, m//128, (g',d)]
            w1v_sb = pp.tile([128, 72], f32)
            b1_sb = pp.tile([128, HEADS], f32)
            uc1bc = pp.tile([128, DIM], f16)
            uc2bc = pp.tile([128, DIM], f16)
            b2bc = pp.tile([128, DIM], f16)
            msk_sb = pp.tile([128, 2], f32)
            ones128 = pp.tile([128, 128], f16)
            b2v = pp.tile([128, DIM], f32)
            w2all = pp.tile([128, HEADS, DIM], f16)

            # ---- PE warmup: junk matmuls fill the input-DMA wait and lift the
            # HAM clock gate to 8/8 before the real stream begins. The tiny
            # copy-out keeps DCE away; warm_out is never read by the host.
            with tc.tile_pool(name="warm", bufs=1) as wp, \
                 tc.tile_pool(name="warmps", bufs=1, space="PSUM") as wps:
                wsrc = wp.tile([128, 512], f16)
                wdst = wp.tile([1, 8], f32)
                nc.vector.memset(wsrc[:], 1.0)
                pw = wps.tile([128, 512], f32)
                for _ in range(28):
                    nc.tensor.matmul(pw[:], wsrc[:, 0:128], wsrc[:],
                                     start=True, stop=True)
                nc.vector.tensor_copy(wdst[:], pw[0:1, 0:8])
                nc.sync.dma_start(warm_out[:], wdst[:])

            nc.sync.dma_start(w1v_sb[:], w1v[:])
            nc.sync.dma_start(b1_sb[:], b1bc[:])
            nc.sync.dma_start(msk_sb[:], msk[:])
            nc.gpsimd.dma_start(b2bc[:], b2blk[0:1, :].to_broadcast((128, DIM)))
            nc.vector.memset(ones128[:], 1.0)

            with tc.tile_pool(name="qk", bufs=1) as qk:
                qT = qk.tile([128, 6, NH], f16)        # [d%128, d//128, n_local]
                kT = qk.tile([128, 6, N], f16)         # [d%128, d//128, m]

                # ---- Phase A: QKV projections (all fp16) --------------------
                with tc.tile_pool(name="pha", bufs=1) as pa, \
                     tc.tile_pool(name="wsl", bufs=4) as wsl, \
                     tc.tile_pool(name="wvr", bufs=1) as wvr, \
                     tc.tile_pool(name="psa", bufs=2, space="PSUM") as psa, \
                     tc.tile_pool(name="psav", bufs=1, space="PSUM") as psav:
                    xf = pa.tile([128, 6, N], f16)
                    xfr = xf_T.rearrange("(c p) n -> p c n", p=128)
                    for d in range(6):
                        nc.gpsimd.dma_start(xf[:, d, 0:NH], xfr[:, d, 0:NH])
                    for d in range(6):
                        nc.gpsimd.dma_start(xf[:, d, NH:N], xfr[:, d, NH:N])
                    wqr = wq_T.rearrange("(c p) n -> p c n", p=128)
                    wkr = wk_T.rearrange("(c p) n -> p c n", p=128)
                    wvr_ap = wv_T.rearrange("(c p) n -> p c n", p=128)

                    for qc in range(6):                # qT[c, n] = sum_d wq[d,c]x[n,d]
                        ps = psa.tile([128, NH], f32, tag="ps512")
                        for d in range(6):
                            wsl_t = wsl.tile([128, 128], f16, tag="w")
                            nc.sync.dma_start(wsl_t[:], wqr[:, d, qc * 128:(qc + 1) * 128])
                            nc.tensor.matmul(ps[:], wsl_t[:], xf[:, d, 0:NH],
                                             start=(d == 0), stop=(d == 5))
                        nc.vector.tensor_copy(qT[:, qc, :], ps[:])
                    for kc in range(6):
                        for mc in range(2):
                            ps = psa.tile([128, 512], f32, tag="ps512")
                            for d in range(6):
                                wsl_t = wsl.tile([128, 128], f16, tag="w")
                                nc.sync.dma_start(wsl_t[:],
                                                  wkr[:, d, kc * 128:(kc + 1) * 128])
                                nc.tensor.matmul(ps[:], wsl_t[:],
                                                 xf[:, d, mc * 512:(mc + 1) * 512],
                                                 start=(d == 0), stop=(d == 5))
                            nc.vector.tensor_copy(kT[:, kc, mc * 512:(mc + 1) * 512],
                                                  ps[:])
                    for vc in range(2):                # v[m, c] = sum_d x[m,d]wv[d,c]
                        wv_t = wvr.tile([128, 6, 384], f16, tag="wv")
                        for d in range(6):
                            nc.sync.dma_start(wv_t[:, d, :],
                                              wvr_ap[:, d, vc * 384:(vc + 1) * 384])
                        for mt in range(8):
                            ps = psa.tile([128, 384], f32, tag="ps384")
                            for d in range(6):
                                nc.tensor.matmul(ps[:], xf[:, d, mt * 128:(mt + 1) * 128],
                                                 wv_t[:, d, :],
                                                 start=(d == 0), stop=(d == 5))
                            nc.vector.tensor_copy(v16[:, mt, vc * 384:(vc + 1) * 384],
                                                  ps[:])
                    # b2v = b2blk * colsum(v), row-replicated via all-ones lhsT
                    psV = psav.tile([128, 2, 512], f32)
                    for half in range(2):
                        for mt in range(8):
                            nc.tensor.matmul(psV[:, half, 0:384],
                                             ones128[:],
                                             v16[:, mt, half * 384:(half + 1) * 384],
                                             start=(mt == 0), stop=(mt == 7))
                    nc.vector.tensor_tensor(b2v[:, 0:384], psV[:, 0, 0:384],
                                            b2bc[:, 0:384], AL.mult)
                    nc.vector.tensor_tensor(b2v[:, 384:768], psV[:, 1, 0:384],
                                            b2bc[:, 384:768], AL.mult)

                # ---- Phase B: mixed scores (mix1 fold), exp, Z, normalize ---
                with tc.tile_pool(name="qsc", bufs=2) as qscp, \
                     tc.tile_pool(name="zbc", bufs=2) as zbcp, \
                     tc.tile_pool(name="zdr", bufs=2, space="DRAM") as zdrp, \
                     tc.tile_pool(name="psb", bufs=4, space="PSUM") as psb, \
                     tc.tile_pool(name="psz", bufs=2, space="PSUM") as psz:
                    for g in range(HEADS):
                        qsc = qscp.tile([128, 6, NH], f16, tag="qsc")
                        for i in range(6):
                            nc.scalar.activation(qsc[:, i, :], qT[:, i, :], AF.Copy,
                                                 scale=w1v_sb[:, g * 6 + i:g * 6 + i + 1])
                        for mt in range(8):
                            ps = psb.tile([128, NH], f32, tag="psb")
                            for i in range(6):
                                nc.tensor.matmul(ps[:], kT[:, i, mt * 128:(mt + 1) * 128],
                                                 qsc[:, i, :], start=(i == 0),
                                                 stop=(i == 5))
                            nc.scalar.activation(E[:, mt, g, :], ps[:], AF.Exp,
                                                 bias=b1_sb[:, g:g + 1], scale=1.0)
                        # Zrow_g[n] = sum_m E_g[m, n]  (free-axis layout, M=1)
                        psZ = psz.tile([1, NH], f32, tag="psz")
                        for mt in range(8):
                            nc.tensor.matmul(psZ[0:1, :], ones128[:, 0:1],
                                             E[:, mt, g, :], start=(mt == 0),
                                             stop=(mt == 7))
                        ztmp = zbcp.tile([1, NH], f32, tag="zt")
                        nc.vector.reciprocal(ztmp[0:1, :], psZ[0:1, :])
                        # broadcast 1/Z to all partitions (fp16) via a DRAM
                        # bounce (SBUF APs cannot have zero partition step)
                        zdr = zdrp.tile([1, NH], f32, tag="zdr")
                        nc.sync.dma_start(zdr[:], ztmp[0:1, :])
                        zb = zbcp.tile([128, NH], f16, tag="zb")
                        nc.gpsimd.dma_start(zb[:], zdr[0:1, :].to_broadcast((128, NH)))
                        for mt in range(8):
                            nc.vector.tensor_tensor(E[:, mt, g, :], E[:, mt, g, :],
                                                    zb[:], AL.mult)

            nc.gpsimd.dma_start(uc1bc[:], uc1[0:1, :].to_broadcast((128, DIM)))
            nc.gpsimd.dma_start(uc2bc[:], uc2[0:1, :].to_broadcast((128, DIM)))
            for g in range(HEADS):
                nc.gpsimd.dma_start(
                    w2all[:, g, :],
                    w2f[0:1, g * DIM:(g + 1) * DIM].to_broadcast((128, DIM)))

            with tc.tile_pool(name="late", bufs=1) as late:
                w1acc = late.tile([128, 4, DIM], f32)
                u16L = late.tile([128, 4, DIM], f16)
                u16R = late.tile([128, 4, DIM], f16)

                # ---- Pass 1: w1 = attn3 @ v (mix2 folded; 8 PSUM bank groups
                #      accumulate over all g and m) ---------------------------
                with tc.tile_pool(name="vt", bufs=2) as vtp, \
                     tc.tile_pool(name="psc", bufs=4, space="PSUM") as psc:
                    psAs = [psc.tile([128, 512], f32, tag="pscA", name=f"pscA{i}") for i in range(4)]
                    psBs = [psc.tile([128, 256], f32, tag="pscB", name=f"pscB{i}") for i in range(4)]
                    for g in range(HEADS):
                        Vt = vtp.tile([128, 8, DIM], f16, tag="vt")
                        for mt in range(8):
                            nc.vector.tensor_tensor(Vt[:, mt, :], v16[:, mt, :],
                                                    w2all[:, g, :], AL.mult)
                        for ns in range(4):
                            for mt in range(8):
                                lhs = E[:, mt, g, ns * 128:(ns + 1) * 128]
                                first = (g == 0 and mt == 0)
                                last = (g == HEADS - 1 and mt == 7)
                                nc.tensor.matmul(psAs[ns][:], lhs, Vt[:, mt, 0:512],
                                                 start=first, stop=last)
                                nc.tensor.matmul(psBs[ns][:], lhs, Vt[:, mt, 512:768],
                                                 start=first, stop=last)
                    for ns in range(4):
                        nc.vector.tensor_tensor(w1acc[:, ns, 0:512], psAs[ns][:],
                                                b2v[:, 0:512], AL.add)
                        nc.vector.tensor_tensor(w1acc[:, ns, 512:768], psBs[ns][:],
                                                b2v[:, 512:768], AL.add)

                # ---- u = (1-2lam)v + 3lam*w1, AllGather u + pass 2 ----------
                # u16L (local rows) is built immediately from w1acc and shipped
                # through the pair AllGather; the partner's block needs no
                # further arithmetic beyond a rank-symmetric mask blend.
                # Pass 2 runs TRANSPOSED: psOut[cb] = [d-block, n] so the
                # output projection consumes it with no PE transposes.
                outT = late.tile([128, 6, NH], f16)
                b2uT = late.tile([128, 6], f32)
                with tc.tile_pool(name="dram", bufs=1, space="DRAM") as dram, \
                     tc.tile_pool(name="w1g", bufs=1) as w1gp, \
                     tc.tile_pool(name="ut", bufs=2) as utp, \
                     tc.tile_pool(name="pse", bufs=6, space="PSUM") as pse, \
                     tc.tile_pool(name="psu", bufs=1, space="PSUM") as psu:
                    u16loc = dram.tile([NH, DIM], f16)
                    u16full = dram.tile([N, DIM], f16)
                    for j in range(4):
                        t1 = w1gp.tile([128, DIM], f16, tag="ub1")
                        t2 = w1gp.tile([128, DIM], f16, tag="ub2")
                        nc.vector.tensor_tensor(t1[:], v16[:, j, :], uc1bc[:], AL.mult)
                        nc.vector.tensor_tensor(t2[:], w1acc[:, j, :], uc2bc[:], AL.mult)
                        nc.vector.tensor_add(u16L[:, j, :], t1[:], t2[:])
                    nc.gpsimd.dma_start(
                        u16loc.rearrange("(ns p) j -> p ns j", p=128), u16L[:])
                    nc.gpsimd.collective_compute(
                        "AllGather", AL.bypass,
                        replica_groups=[[0, 1], [2, 3], [4, 5], [6, 7]],
                        ins=[u16loc.opt()], outs=[u16full.opt()])
                    psOut = [pse.tile([128, 512], f32, tag="psO", name=f"psO{i}")
                             for i in range(6)]
                    # local half of pass 2: no collective dependency
                    for g in range(HEADS):
                        UtL = utp.tile([128, 4, DIM], f16, tag="ut")
                        for j in range(4):
                            nc.vector.tensor_tensor(UtL[:, j, :], u16L[:, j, :],
                                                    w2all[:, g, :], AL.mult)
                        for cb in range(6):
                            for j in range(4):
                                nc.tensor.matmul(
                                    psOut[cb][:],
                                    UtL[:, j, cb * 128:(cb + 1) * 128],
                                    E[:, j, g, :],
                                    start=(g == 0 and j == 0), stop=False)
                    # remote half: mask-blend of the two gathered blocks
                    # (in-place scale of each block, then add)
                    u16fr = u16full.rearrange("(blk p) j -> p blk j", p=128)
                    u16b = w1gp.tile([128, 2, 4, DIM], f16)
                    nc.sync.dma_start(u16b[:, 0, :, :], u16fr[:, 0:4, :])
                    nc.sync.dma_start(u16b[:, 1, :, :], u16fr[:, 4:8, :])
                    nc.vector.tensor_scalar(u16b[:, 0, :, :], u16b[:, 0, :, :],
                                            msk_sb[:, 0:1], None, AL.mult)
                    nc.vector.tensor_scalar(u16b[:, 1, :, :], u16b[:, 1, :, :],
                                            msk_sb[:, 1:2], None, AL.mult)
                    nc.vector.tensor_add(u16R[:], u16b[:, 0, :, :],
                                         u16b[:, 1, :, :])
                    # colsum(u) for the attn3 bias term, transposed to [d, 1]
                    # via a DRAM bounce so the pass-2 consume can add it as a
                    # per-partition scalar.
                    psU = psu.tile([128, 2, 512], f32)
                    for j in range(4):
                        nc.tensor.matmul(psU[:, 0, :], ones128[:], u16L[:, j, 0:512],
                                         start=(j == 0), stop=False)
                        nc.tensor.matmul(psU[:, 1, 0:256], ones128[:],
                                         u16L[:, j, 512:768],
                                         start=(j == 0), stop=False)
                    for j in range(4):
                        nc.tensor.matmul(psU[:, 0, :], ones128[:], u16R[:, j, 0:512],
                                         start=False, stop=(j == 3))
                        nc.tensor.matmul(psU[:, 1, 0:256], ones128[:],
                                         u16R[:, j, 512:768],
                                         start=False, stop=(j == 3))
                    b2u = w1gp.tile([128, DIM], f32)
                    nc.vector.tensor_tensor(b2u[:, 0:512], psU[:, 0, :],
                                            b2bc[:, 0:512], AL.mult)
                    nc.vector.tensor_tensor(b2u[:, 512:768], psU[:, 1, 0:256],
                                            b2bc[:, 512:768], AL.mult)
                    b2udr = dram.tile([1, DIM], f32)
                    nc.sync.dma_start(b2udr[:], b2u[0:1, :])
                    nc.gpsimd.dma_start(
                        b2uT[:], b2udr.rearrange("o (c p) -> p (c o)", p=128))

                    for g in range(HEADS):
                        UtR = utp.tile([128, 4, DIM], f16, tag="ut")
                        for j in range(4):
                            nc.vector.tensor_tensor(UtR[:, j, :], u16R[:, j, :],
                                                    w2all[:, g, :], AL.mult)
                        for cb in range(6):
                            for j in range(4):
                                nc.tensor.matmul(
                                    psOut[cb][:],
                                    UtR[:, j, cb * 128:(cb + 1) * 128],
                                    E[:, 4 + j, g, :],
                                    start=False,
                                    stop=(g == HEADS - 1 and j == 3))

                    # pass-2 consume: add the attn3-bias term (per-partition in
                    # the transposed layout), straight into outT
                    for cb in range(6):
                        nc.vector.tensor_scalar(outT[:, cb, :], psOut[cb][:],
                                                b2uT[:, cb:cb + 1], None, AL.add)

                # ---- Phase F: output projection (outT fed directly) ---------
                with tc.tile_pool(name="phf", bufs=1) as pf, \
                     tc.tile_pool(name="wos", bufs=6) as wos, \
                     tc.tile_pool(name="ypool", bufs=2) as ypool:
                    obbc = pf.tile([128, DIM], f16)
                    nc.gpsimd.dma_start(obbc[:], ob[0:1, :].to_broadcast((128, DIM)))
                    wor = wo_T.rearrange("(c p) n -> p c n", p=128)
                    wo_ts = []
                    for jc in range(6):
                        wo_t = wos.tile([128, DIM], f16, tag="wo", name=f"wo{jc}")
                        nc.gpsimd.dma_start(wo_t[:], wor[:, jc, :])
                        wo_ts.append(wo_t)
                    yr = y.rearrange("(ns p) j -> p ns j", p=128)
                    with tc.tile_pool(name="psf", bufs=2, space="PSUM") as psf:
                        for ns in range(4):
                            psY = psf.tile([128, 512], f32, tag="psY")
                            psY2 = psf.tile([128, 512], f32, tag="psY2")
                            for jc in range(6):
                                nc.tensor.matmul(psY[:, :],
                                                 outT[:, jc, ns * 128:(ns + 1) * 128],
                                                 wo_ts[jc][:, 0:512], start=(jc == 0),
                                                 stop=(jc == 5))
                                nc.tensor.matmul(psY2[:, 0:256],
                                                 outT[:, jc, ns * 128:(ns + 1) * 128],
                                                 wo_ts[jc][:, 512:768], start=(jc == 0),
                                                 stop=(jc == 5))
                            y_sb = ypool.tile([128, DIM], f32, tag="ysb")
                            nc.vector.tensor_tensor(y_sb[:, 0:512], psY[:, :],
                                                    obbc[:, 0:512], AL.add)
                            nc.vector.tensor_tensor(y_sb[:, 512:768], psY2[:, 0:256],
                                                    obbc[:, 512:768], AL.add)
                            nc.sync.dma_start(yr[:, ns, :], y_sb[:])

    nc.compile()
    return nc


def kernel(x, qkv_w, proj_l_w, proj_l_b, proj_w_w, proj_w_b, lamb,
           proj_out_w, proj_out_b):
    x = np.asarray(x, dtype=np.float32)
    qkv_w = np.asarray(qkv_w, dtype=np.float32)
    proj_l_w = np.asarray(proj_l_w, dtype=np.float32)
    proj_l_b = np.asarray(proj_l_b, dtype=np.float32)
    proj_w_w = np.asarray(proj_w_w, dtype=np.float32)
    proj_w_b = np.asarray(proj_w_b, dtype=np.float32)
    lamb = np.asarray(lamb, dtype=np.float32)
    proj_out_w = np.asarray(proj_out_w, dtype=np.float32)
    proj_out_b = np.asarray(proj_out_b, dtype=np.float32)

    nc = _build()

    wq_T = (np.ascontiguousarray(qkv_w[:DIM].T) * np.float32(SCALE)).astype(np.float16)
    wk_T = np.ascontiguousarray(qkv_w[DIM:2 * DIM].T).astype(np.float16)
    wv_T = np.ascontiguousarray(qkv_w[2 * DIM:].T).astype(np.float16)
    wo_T = np.ascontiguousarray(proj_out_w.T).astype(np.float16)

    w1v = np.empty((128, 72), dtype=np.float32)
    for g in range(HEADS):
        for i in range(6):
            w1v[:64, g * 6 + i] = proj_l_w[g, 2 * i]
            w1v[64:, g * 6 + i] = proj_l_w[g, 2 * i + 1]
    b1bc = np.tile(proj_l_b[None, :], (128, 1)).astype(np.float32)
    # w2f[0, g*768 + g'*64 + d] = proj_w_w[g', g]
    w2f = np.repeat(proj_w_w.T, HD, axis=1).reshape(1, HEADS * DIM).astype(np.float32)
    uc1 = np.repeat(1.0 - 2.0 * lamb, HD)[None, :].astype(np.float32)
    uc2 = np.repeat(3.0 * lamb, HD)[None, :].astype(np.float32)
    b2blk = np.repeat(proj_w_b, HD)[None, :].astype(np.float32)
    ob = proj_out_b[None, :].astype(np.float32)

    in_maps = []
    for c in range(8):
        b, half = c // 2, c % 2
        # m-axis rotated: rows [0:512] are this core's own query rows
        xr = np.concatenate([x[b, half * NH:(half + 1) * NH, :],
                             x[b, (1 - half) * NH:(2 - half) * NH, :]], axis=0)
        mskv = np.empty((128, 2), dtype=np.float32)
        mskv[:, 0] = float(half)        # weight for gathered block 0 (= rank 0)
        mskv[:, 1] = float(1 - half)    # weight for gathered block 1 (= rank 1)
        in_maps.append({
            "xf_T": np.ascontiguousarray(xr.T).astype(np.float16),
            "wq_T": wq_T, "wk_T": wk_T, "wv_T": wv_T, "wo_T": wo_T,
            "w1v": w1v, "b1bc": b1bc, "w2f": w2f,
            "uc1": uc1, "uc2": uc2, "b2blk": b2blk, "ob": ob, "msk": mskv,
        })

    res = run_bass_kernel_spmd(nc, in_maps, core_ids=list(range(8)),
                               trace=TRACE, **TRACE_KW)
    kernel.last_results = res
    kernel.last_nc = nc
    kernel.last_in_maps = in_maps

    out = np.empty((B, N, DIM), dtype=np.float32)
    for c in range(8):
        b, half = c // 2, c % 2
        out[b, half * NH:(half + 1) * NH, :] = res.results[c]["y"]
    return out
